# revision 43
# baseline (speedup 1.0000x reference)
"""Trainium2 Bass kernel for nn_LAINRDecoder (sparse attention INR decoder).

Strategy
--------
Per query q (identical for all batches) the reference computes
  idx = f(grid row); bias = ALPHA*(idx/N - tok_pos)^2; top-128 smallest bias.
The top-128 set is the CONTIGUOUS token window [s, s+128) with
s = clip(floor((idx+1)/4) - 64, 0, 896)  (tie-breaking of jax.lax.top_k
derived analytically; test.py re-verifies against the reference each run).
Softmax attention over a gathered set == dense attention over a token range
with a per-query window mask.

Host-side preprocessing (all cheap numpy):
  * compute s per query, argsort queries by s, shard 512 sorted queries/core
  * each core then only needs a CONTIGUOUS token window of CC 128-chunks
    (CC ~ 4 instead of all 8), identical program on every core (SPMD)
  * window masks are precomputed on host in the exact (token, query) layout
    the kernel consumes; tokens/weights are pre-transposed and pre-cast to
    bf16 so the device does zero transposes and zero index arithmetic
  * outputs are inverse-permuted back on host

Device kernel (per core, everything transposed: features on partitions,
queries on the free dim):
  * gamma features with f32 Cody-Waite range reduction + Sin LUT
  * K^T/V built from pre-transposed tokens (bf16 matmuls, f32 accum)
  * ST = K^T q per (query-tile, batch, head) over CC chunks; heads live in
    partition ranges 0:64 / 64:128 so head pairs run as concurrent PE
    row-tiles; exp (scalar LUT) -> mask multiply (DVE, host mask)
  * AV with a ones-augmented V so the softmax denominator falls out of the
    same matmul; 1/denom via the fast DVE reciprocal; broadcast via a
    1-contraction PE matmul
  * MLP tail on [b0|b1]-packed 256-wide tiles, relu+bias folded into
    two-op tensor_scalar instructions, h_l added via identity-matmul
    accumulation into PSUM
"""

import math
import os
import sys
import types
from contextlib import ExitStack

import numpy as np
import ml_dtypes

BF16 = ml_dtypes.bfloat16


# ---------------------------------------------------------------------------
# environment shims (axon NTFF hook + artifact upload are absent in this
# container; inject them so run_bass_kernel_spmd works with trace=True)
# ---------------------------------------------------------------------------
def _install_shims():
    if "antenv.axon_hooks" not in sys.modules:
        hooks = types.ModuleType("antenv.axon_hooks")
        try:
            from trn_agent_boot.trn_boot import _ntff_profile_via_ctypes

            _hook = _ntff_profile_via_ctypes("/opt/axon/libaxon_pjrt.so")
        except Exception:
            _hook = None
        hooks.get_axon_ntff_profile_hook = lambda: _hook
        hooks.set_axon_ntff_profile_hook = lambda h: None
        sys.modules["antenv.axon_hooks"] = hooks
    import concourse.bass_utils as bass_utils

    bass_utils.upload_artifacts = lambda tmpdir: tmpdir


_install_shims()

import concourse.bass as bass
import concourse.mybir as mybir
import concourse.tile as tile
from concourse.bass_utils import run_bass_kernel_spmd
from concourse.masks import make_identity

F32 = mybir.dt.float32
I32 = mybir.dt.int32
BF = mybir.dt.bfloat16
AF = mybir.ActivationFunctionType
OP = mybir.AluOpType

# problem constants (hardcoded per the harness contract)
B = 2
Q = 4096
L = 1024
HD = 256
FD = 64
INNER = 128
HEADS = 2
DH = 64
TOPK = 128
N_FREQ = 8
LAYER_NUM = 2
N_CORES = 8
QS = Q // N_CORES          # queries per core (512)
QT = 128                   # query tile
NQT = QS // QT             # query tiles per core (4)
SCALE = DH ** -0.5

TWO_PI = 2.0 * math.pi
# Cody-Waite split of 2*pi: hi has <=13 mantissa bits so k*hi (k<=64) is exact
TWO_PI_HI = float(np.float32(np.ldexp(np.round(np.ldexp(TWO_PI, 11)), -11)))
TWO_PI_LO = float(np.float32(TWO_PI - TWO_PI_HI))


def _omegas(sigma):
    return np.logspace(1.0, np.log10(sigma), N_FREQ).astype(np.float32)


def _w2(sigma):
    """(4, 64) matrix: arg[q, c*16+j] (j<8 sin slot, j>=8 cos slot) = pi*omega_j*grid[q,c]."""
    w = np.zeros((4, 64), np.float32)
    om = _omegas(sigma)
    for c in range(4):
        for j in range(N_FREQ):
            w[c, c * 16 + j] = np.float32(math.pi) * om[j]
            w[c, c * 16 + 8 + j] = np.float32(math.pi) * om[j]
    return w


def _sincos_bias():
    """(64,1) activation bias: 0 for sin rows, pi/2 for cos rows."""
    b = np.zeros((64, 1), np.float32)
    for c in range(4):
        b[c * 16 + 8 : c * 16 + 16, 0] = np.float32(math.pi / 2)
    return b


def build_program(CC):
    """CC = token chunks (of 128) in each core's window."""
    CL = CC * 128
    nc = bass.Bass("TRN2", target_bir_lowering=False, debug=False)

    def din(name, shape, dt=F32):
        return nc.dram_tensor(name, shape, dt, kind="ExternalInput").ap()

    # packed input blobs — one DMA each (50 small DMAs serialized ~30us of
    # head latency before the first matmul in the unpacked version)
    x0sT = din("x0sT", (4, QS))
    cb64 = din("cb64", (4, 128))             # [w2q | w2b] stacked on free dim
    scb2 = din("scb2", (128, 1))             # sin/cos phase bias, both bands
    bblob = din("bblob", (128, 15))          # all (128,1) biases + olb
    wb64 = din("wb64", (128, 768), BF)       # qryW|bandW0 (rows 0:64), bandW1 (rows 64:128)
    tokT = din("tokT", (B, 2, 128, CL), BF)
    wblob = din("wblob", (128, 2564), BF)    # qW|kvWk|kvWv|outW|modW|hvW|olW
    maskT = din("maskT", (128, NQT * CL), BF)
    out_d = nc.dram_tensor("out", (B, QS), F32, kind="ExternalOutput").ap()

    ctx = ExitStack()
    with tile.TileContext(nc) as tc:
        cpool = ctx.enter_context(tc.tile_pool(name="consts", bufs=1))
        wpool = ctx.enter_context(tc.tile_pool(name="weights", bufs=1))
        featp = ctx.enter_context(tc.tile_pool(name="feat", bufs=1))
        kvp = ctx.enter_context(tc.tile_pool(name="kv", bufs=1))
        mkp = ctx.enter_context(tc.tile_pool(name="mask", bufs=1))
        ep = ctx.enter_context(tc.tile_pool(name="ep", bufs=4))
        pmp = ctx.enter_context(tc.tile_pool(name="pm", bufs=8))
        invp = ctx.enter_context(tc.tile_pool(name="inv", bufs=4))
        bcsp = ctx.enter_context(tc.tile_pool(name="bcs", bufs=4))
        onp = ctx.enter_context(tc.tile_pool(name="on", bufs=3))
        mlt = ctx.enter_context(tc.tile_pool(name="mlt", bufs=3))
        orp = ctx.enter_context(tc.tile_pool(name="orow", bufs=4))
        p_st = ctx.enter_context(tc.tile_pool(name="pst", bufs=3, space="PSUM"))
        p_pot = ctx.enter_context(tc.tile_pool(name="ppot", bufs=2, space="PSUM"))
        p_ml = ctx.enter_context(tc.tile_pool(name="pml", bufs=3, space="PSUM"))

        # ---- inputs: feature-path DMAs first, then tokens/weights/masks
        gridT = featp.tile([4, QS], F32, tag="gridT", name="gridT")
        nc.sync.dma_start(gridT[:], x0sT[:])
        t_cb = cpool.tile([4, 128], F32, tag="cb64", name="cb64")
        nc.sync.dma_start(t_cb[:], cb64[:])
        t_scb = cpool.tile([128, 1], F32, tag="scb2", name="scb2")
        nc.sync.dma_start(t_scb[:], scb2[:])
        t_bb = wpool.tile([128, 15], F32, tag="bblob", name="bblob")
        nc.sync.dma_start(t_bb[:], bblob[:])
        t_w64 = wpool.tile([128, 768], BF, tag="wb64", name="wb64")
        nc.sync.dma_start(t_w64[:], wb64[:])
        ttk = [[kvp.tile([128, CL], BF, tag=f"ttk{b}{k}", name=f"ttk{b}{k}")
                for k in range(2)] for b in range(B)]
        for b in range(B):
            for k in range(2):
                nc.sync.dma_start(ttk[b][k][:], tokT[b, k])
        t_wb = wpool.tile([128, 2564], BF, tag="wblob", name="wblob")
        nc.sync.dma_start(t_wb[:], wblob[:])
        t_mask = mkp.tile([128, NQT * CL], BF, tag="mask", name="mask")
        nc.sync.dma_start(t_mask[:], maskT[:])

        # blob slices (views, no copies)
        t_qb = [t_bb[:, i : i + 1] for i in range(2)]
        t_outb = [t_bb[:, 2 + i : 3 + i] for i in range(2)]
        t_bandb = [[t_bb[:, 4 + 2 * l + i : 5 + 2 * l + i] for i in range(2)]
                   for l in range(2)]
        t_modb = [[t_bb[:, 8 + 2 * l + i : 9 + 2 * l + i] for i in range(2)]
                  for l in range(2)]
        t_hvb = [t_bb[:, 12 + i : 13 + i] for i in range(2)]
        t_olb = t_bb[0:1, 14:15]
        t_qryW = t_w64[0:64, 0:256]
        t_bandW = [t_w64[0:64, 256:512], t_w64[64:128, 512:768]]
        t_qW = [t_wb[:, 128 * k : 128 * k + 128] for k in range(2)]
        t_kvWk = [t_wb[:, 256 + 128 * k : 384 + 128 * k] for k in range(2)]
        t_kvWv = [t_wb[:, 512 + 128 * k : 640 + 128 * k] for k in range(2)]
        t_outW = t_wb[:, 768:1024]
        t_modW = [[t_wb[:, 1024 + 512 * l + 256 * k : 1280 + 512 * l + 256 * k]
                   for k in range(2)] for l in range(2)]
        t_hvW = [t_wb[:, 2048 + 256 * k : 2304 + 256 * k] for k in range(2)]
        t_olW = [t_wb[:, 2560 + 2 * k : 2562 + 2 * k] for k in range(2)]

        # ---- constants -------------------------------------------------
        identb = cpool.tile([128, 128], BF, tag="identb", name="identb")
        make_identity(nc, identb[:])
        ones1 = cpool.tile([1, 128], BF, tag="ones1", name="ones1")
        nc.vector.memset(ones1[:], 1.0)

        # PE warm-up + heaters: the HAM clock gate keeps the PE at 1.2 GHz
        # until it has seen ~3.4us of sustained matmul activity, and it
        # re-throttles whenever a 3.4us window has low duty.  The initial
        # burst warms the PE during the input-DMA window; the small heater
        # groups sprinkled through the dependency-stall-heavy feature/KV
        # phase hold the gate open (each group runs while the next real
        # matmul is waiting on its semaphore, so they cost ~nothing).
        wscr = p_st.tile([128, 512], F32, tag="pst", name="warm")
        _hix = [0]

        def heater(n):
            for _ in range(n):
                i = _hix[0] = _hix[0] + 1
                nc.tensor.matmul(wscr[:, (i % 4) * 128 : (i % 4 + 1) * 128],
                                 identb[:], identb[:], start=True, stop=True)

        heater(40)

        # ---- gamma features (f32 range reduction, Sin LUT first) -------
        # both bands' gamma args stacked on partitions: rows 0:64 sigma=128
        # (attention + band 0), rows 64:128 sigma=32 (band 1)
        pa = p_st.tile([128, 512], F32, tag="pst", name="pa_g")
        nc.tensor.matmul(pa[:], t_cb[:], gridT[:], start=True, stop=True)
        a1 = featp.tile([128, QS], F32, tag="g_a1", name="g_a1")
        nc.vector.tensor_scalar(a1[:], pa[:], t_scb[:], None, OP.add)
        u = featp.tile([128, QS], F32, tag="g_u", name="g_u")
        nc.vector.tensor_scalar(u[:], a1[:], 1.0 / TWO_PI, None, OP.mult)
        ki = featp.tile([128, QS], I32, tag="g_ki", name="g_ki")
        nc.vector.tensor_copy(ki[:], u[:])  # round-to-nearest
        kf = featp.tile([128, QS], F32, tag="g_kf", name="g_kf")
        nc.vector.tensor_copy(kf[:], ki[:])
        nc.vector.tensor_scalar(u[:], kf[:], TWO_PI_HI, None, OP.mult)
        nc.vector.tensor_tensor(a1[:], a1[:], u[:], OP.subtract)
        nc.vector.tensor_scalar(u[:], kf[:], TWO_PI_LO, None, OP.mult)
        nc.vector.tensor_tensor(a1[:], a1[:], u[:], OP.subtract)
        g2 = featp.tile([128, QS], BF, tag="g2", name="g2")
        nc.scalar.activation(g2[:], a1[:], AF.Sin)
        heater(10)
        gq = g2[0:64, :]
        gb1 = g2[64:128, :]

        # x_qT (2 x (128, 512) bf16) = relu(query_W^T @ gamma + qb)
        x_qT = []
        for i in range(2):
            px = p_st.tile([128, 512], F32, tag="pst", name=f"pxq{i}")
            nc.tensor.matmul(px[:], t_qryW[:, i * 128 : (i + 1) * 128], gq,
                             start=True, stop=True)
            xq = featp.tile([128, QS], BF, tag=f"xq{i}", name=f"xq{i}")
            nc.scalar.activation(xq[:], px[:], AF.Relu, bias=t_qb[i][:])
            x_qT.append(xq)
            heater(4)
        # qT (128, 512) bf16, pre-scaled
        pq = p_st.tile([128, 512], F32, tag="pst", name="pq")
        for k in range(2):
            nc.tensor.matmul(pq[:], t_qW[k][:], x_qT[k][:], start=(k == 0), stop=(k == 1))
        qTt = featp.tile([INNER, QS], BF, tag="qTt", name="qTt")
        nc.scalar.activation(qTt[:], pq[:], AF.Copy, scale=SCALE)
        heater(4)
        # band features h_lT
        h_lT = [[None, None], [None, None]]
        for l, gsrc in ((0, gq), (1, gb1)):  # APs, not tiles
            for mc in range(2):
                ph = p_st.tile([128, 512], F32, tag="pst", name=f"ph{l}{mc}")
                nc.tensor.matmul(ph[:], t_bandW[l][:, mc * 128 : (mc + 1) * 128],
                                 gsrc, start=True, stop=True)
                hl = featp.tile([128, QS], BF, tag=f"hl{l}{mc}", name=f"hl{l}{mc}")
                nc.scalar.activation(hl[:], ph[:], AF.Relu, bias=t_bandb[l][mc][:])
                h_lT[l][mc] = hl
                heater(4)

        # ---- K^T and ones-augmented V per batch ------------------------
        t_KT = [kvp.tile([128, CL], BF, tag=f"KT{b}", name=f"KT{b}") for b in range(B)]
        t_V = [kvp.tile([128, CC * 130], BF, tag=f"V{b}", name=f"V{b}") for b in range(B)]
        for b in range(B):
            nc.vector.memset(t_V[b][:], 1.0)  # ones columns survive the copies
            for g0 in range(0, CC, 4):
                gn = min(4, CC - g0)
                pk = p_st.tile([128, 512], F32, tag="pst", name=f"pk{b}{g0}")
                for cc in range(gn):
                    c = g0 + cc
                    for k in range(2):
                        nc.tensor.matmul(
                            pk[:, cc * 128 : (cc + 1) * 128], t_kvWk[k][:],
                            ttk[b][k][:, c * 128 : (c + 1) * 128],
                            start=(k == 0), stop=(k == 1))
                nc.any.tensor_copy(t_KT[b][:, g0 * 128 : (g0 + gn) * 128],
                                   pk[:, : gn * 128])
                heater(4)
            for c in range(CC):
                pv = p_ml.tile([128, 256], F32, tag="pml", name=f"pv{b}{c}")
                for k in range(2):
                    nc.tensor.matmul(pv[:, 0:128], ttk[b][k][:, c * 128 : (c + 1) * 128],
                                     t_kvWv[k][:], start=(k == 0), stop=(k == 1))
                nc.any.tensor_copy(t_V[b][:, c * 130 : c * 130 + 64], pv[:, 0:64])
                nc.any.tensor_copy(t_V[b][:, c * 130 + 65 : c * 130 + 129], pv[:, 64:128])

        # ---- attention + MLP tail per query tile -----------------------
        oNqs = [onp.tile([128, 256], BF, tag=f"oNq{qt}", name=f"oNq{qt}")
                for qt in range(NQT)]
        for qt in range(NQT):
            oNq = oNqs[qt]
            mk = t_mask[:, qt * CL : (qt + 1) * CL]
            for b in range(B):
                # AV with queries on PARTITIONS (lhsT = masked probabilities)
                # so the softmax denominator is a per-partition scalar: the
                # normalization is then 2 tiny reciprocals + 2 scaled copies
                # + one PE transpose instead of Ln/Exp/broadcast-matmul.
                potq = p_pot.tile([128, 130], F32, tag="pot", name=f"pot{qt}{b}")
                for h in range(2):
                    pst = p_st.tile([128, 512], F32, tag="pst", name=f"pst{qt}{b}{h}")
                    for c in range(CC):
                        nc.tensor.matmul(
                            pst[:, c * 128 : (c + 1) * 128],
                            t_KT[b][h * 64 : h * 64 + 64, c * 128 : (c + 1) * 128],
                            qTt[h * 64 : h * 64 + 64, qt * 128 : (qt + 1) * 128],
                            start=True, stop=True, tile_position=(h * 64, 0))
                    e = ep.tile([128, CL], BF, tag="e", name=f"e{qt}{b}{h}")
                    nc.scalar.activation(e[:], pst[:, :CL], AF.Exp)
                    pm = pmp.tile([128, CL], BF, tag="pm", name=f"pm{qt}{b}{h}")
                    nc.vector.tensor_tensor(pm[:], e[:], mk, OP.mult)
                    if qt < 2:
                        heater(2)
                    for c in range(CC):
                        nc.tensor.matmul(
                            potq[:, h * 65 : (h + 1) * 65],
                            pm[:, c * 128 : (c + 1) * 128],
                            t_V[b][:, c * 130 + h * 65 : c * 130 + (h + 1) * 65],
                            start=(c == 0), stop=(c == CC - 1))
                invh = invp.tile([128, 2], F32, tag="invh", name=f"invh{qt}{b}")
                onT = bcsp.tile([128, 128], BF, tag="onT", name=f"onT{qt}{b}")
                for h in range(2):
                    nc.vector.reciprocal(invh[:, h : h + 1],
                                         potq[:, h * 65 + 64 : h * 65 + 65])
                    nc.scalar.activation(onT[:, h * 64 : (h + 1) * 64],
                                         potq[:, h * 65 : h * 65 + 64],
                                         AF.Copy, scale=invh[:, h : h + 1])
                ptr = p_ml.tile([128, 128], BF, tag="pml", name=f"ptr{qt}{b}")
                nc.tensor.transpose(ptr[:], onT[:], identb[:])
                nc.any.tensor_copy(oNq[:, b * 128 : (b + 1) * 128], ptr[:])

            # ---- MLP tail, free dim 256 = [b0 | b1] --------------------
            qsl = slice(qt * 128, (qt + 1) * 128)
            mt = []
            for mc in range(2):
                pm0 = p_ml.tile([128, 256], F32, tag="pml", name=f"pmod{qt}{mc}")
                nc.tensor.matmul(pm0[:], t_outW[:, mc * 128 : (mc + 1) * 128],
                                 oNq[:], start=True, stop=True)
                m = mlt.tile([128, 256], BF, tag=f"mt{mc}", name=f"mt{qt}{mc}")
                nc.any.tensor_scalar(m[:], pm0[:], t_outb[mc][:], None, OP.add)
                mt.append(m)
            mls = [[None, None], [None, None]]
            for l in range(2):
                for mc in range(2):
                    pm0 = p_ml.tile([128, 256], F32, tag="pml", name=f"pml{qt}{l}{mc}")
                    for k in range(2):
                        nc.tensor.matmul(pm0[:], t_modW[l][k][:, mc * 128 : (mc + 1) * 128],
                                         mt[k][:], start=(k == 0), stop=False)
                    hsl = h_lT[l][mc][:, qsl]
                    nc.tensor.matmul(pm0[:, 0:128], identb[:], hsl, start=False, stop=False)
                    nc.tensor.matmul(pm0[:, 128:256], identb[:], hsl, start=False, stop=True)
                    ml = mlt.tile([128, 256], BF, tag=f"ml{l}{mc}", name=f"ml{qt}{l}{mc}")
                    nc.any.tensor_scalar(ml[:], pm0[:], t_modb[l][mc][:], 0.0,
                                         OP.add, OP.max)
                    mls[l][mc] = ml
            s01 = []
            for mc in range(2):
                s0 = mlt.tile([128, 256], BF, tag=f"s01{mc}", name=f"s01{qt}{mc}")
                nc.any.tensor_tensor(s0[:], mls[0][mc][:], mls[1][mc][:], OP.add)
                s01.append(s0)
            hv1 = []
            for mc in range(2):
                pm0 = p_ml.tile([128, 256], F32, tag="pml", name=f"phv{qt}{mc}")
                for k in range(2):
                    nc.tensor.matmul(pm0[:], t_hvW[k][:, mc * 128 : (mc + 1) * 128],
                                     s01[k][:], start=(k == 0), stop=(k == 1))
                hv = mlt.tile([128, 256], BF, tag=f"hv{mc}", name=f"hv{qt}{mc}")
                nc.any.tensor_scalar(hv[:], pm0[:], t_hvb[mc][:], 0.0, OP.add, OP.max)
                hv1.append(hv)
            por = p_ml.tile([1, 256], F32, tag="pml", name=f"por{qt}")
            steps = [(t_olW[k][:, 0:1], mls[0][k]) for k in range(2)] + \
                    [(t_olW[k][:, 1:2], hv1[k]) for k in range(2)]
            for si, (lw, rv) in enumerate(steps):
                nc.tensor.matmul(por[:], lw, rv[:], start=(si == 0),
                                 stop=(si == len(steps) - 1))
            orow = orp.tile([1, 256], F32, tag="orow", name=f"orow{qt}")
            nc.any.tensor_scalar(orow[:], por[:], t_olb[:], None, OP.add)
            for b in range(B):
                nc.sync.dma_start(out_d[b : b + 1, qt * 128 : (qt + 1) * 128],
                                  orow[:, b * 128 : (b + 1) * 128])
        ctx.close()

    _split_multi_waits_inline(nc)
    return nc


def _split_multi_waits_inline(nc):
    """Self-contained copy of the wait-splitting post-pass."""
    for fn in nc.m.functions:
        for blk in fn.blocks:
            new_insts = []
            for inst in blk.instructions:
                si = getattr(inst, "sync_info", None)
                if si is not None and len(si.on_wait) > 1:
                    waits = list(si.on_wait)
                    for j, w in enumerate(waits[:-1]):
                        new_insts.append(mybir.InstNoOp(
                            name=f"{inst.name}-ws{j}",
                            engine=inst.engine,
                            sync_info=mybir.SyncInfo(on_wait=[w], on_update=[]),
                            bass_nofuse=True,
                        ))
                    si.on_wait = waits[-1:]
                new_insts.append(inst)
            blk.instructions = new_insts


_CACHED = {}
LAST_RESULTS = None


def _bf(a):
    return np.ascontiguousarray(np.asarray(a).astype(BF16))


def kernel(**inputs):
    global LAST_RESULTS
    x = np.asarray(inputs["x"], np.float32)
    tokens = np.asarray(inputs["tokens"], np.float32)
    assert int(inputs["gD"]) == 8 and int(inputs["gH"]) == 8
    assert int(inputs["gW"]) == 8 and int(inputs["gT"]) == 8

    grid = x[0]  # (Q, 4) — reference uses x[0] for all batches
    # window start per query, matching reference f32->int32 truncation exactly
    z = (grid[:, 0] * np.float32(8)).astype(np.int32).astype(np.int64)
    y = (grid[:, 1] * np.float32(8)).astype(np.int32).astype(np.int64)
    xx = (grid[:, 2] * np.float32(8)).astype(np.int32).astype(np.int64)
    t = (grid[:, 3] * np.float32(8)).astype(np.int32).astype(np.int64)
    idx = ((t * 8 + z) * 8 + y) * 8 + xx
    s = np.clip((idx + 1) // 4 - 64, 0, L - TOPK)          # (Q,)

    perm = np.argsort(s, kind="stable")
    s_sorted = s[perm]
    grid_sorted = grid[perm]

    # per-core contiguous token windows; the base token offset is arbitrary
    # (NOT 128-aligned) — masks and host-side slicing absorb the offset, so
    # each core needs only ceil(span/128) chunks (2 for uniform data)
    t0s, ccs = [], []
    for c in range(N_CORES):
        sc = s_sorted[c * QS : (c + 1) * QS]
        t0 = int(sc.min())
        span = int(sc.max()) + TOPK - t0
        t0s.append(t0)
        ccs.append(-(-span // 128))                        # ceil
    CC = min(max(ccs), L // 128)
    t0s = [min(t0, L - CC * 128) for t0 in t0s]
    for c in range(N_CORES):
        sc = s_sorted[c * QS : (c + 1) * QS]
        assert int(sc.min()) >= t0s[c]
        assert int(sc.max()) + TOPK <= t0s[c] + CC * 128

    if CC not in _CACHED:
        _CACHED[CC] = build_program(CC)
    nc = _CACHED[CC]
    CL = CC * 128

    def r2(a):
        return np.ascontiguousarray(np.asarray(a, np.float32).reshape(-1, 2, 128, 1))

    kv_W = np.asarray(inputs["kv_W"], np.float32)
    q_W = np.asarray(inputs["q_W"], np.float32)
    mod_W = np.asarray(inputs["mod_W"], np.float32)
    hv_W = np.asarray(inputs["hv_W"], np.float32)[0]
    outl_W = np.asarray(inputs["outl_W"], np.float32)      # (2, 256, 1)
    olw = np.stack([np.stack([outl_W[l, k * 128 : (k + 1) * 128, 0] for l in range(2)],
                             axis=1) for k in range(2)])   # (2, 128, 2)

    # packed blobs (layouts must match the slice offsets in build_program)
    qW2 = q_W.reshape(2, 128, 128)
    kvk = kv_W.reshape(2, 128, 256)[:, :, :128]
    kvv = kv_W.reshape(2, 128, 256)[:, :, 128:]
    modW4 = mod_W.reshape(2, 2, 128, 256)
    wblob = np.concatenate(
        [qW2[0], qW2[1], kvk[0], kvk[1], kvv[0], kvv[1],
         np.asarray(inputs["out_W"], np.float32),
         modW4[0, 0], modW4[0, 1], modW4[1, 0], modW4[1, 1],
         hv_W.reshape(2, 128, 256)[0], hv_W.reshape(2, 128, 256)[1],
         olw[0], olw[1]], axis=1)                       # (128, 2564)
    bW = np.asarray(inputs["band_W"], np.float32)
    wb64 = np.zeros((128, 768), np.float32)
    wb64[0:64, 0:256] = np.asarray(inputs["query_W"], np.float32)
    wb64[0:64, 256:512] = bW[0]
    wb64[64:128, 512:768] = bW[1]
    cb64 = np.concatenate([_w2(128.0), _w2(32.0)], axis=1)   # (4, 128)
    scb2 = np.concatenate([_sincos_bias()] * 2, axis=0)       # (128, 1)
    bb = np.zeros((128, 15), np.float32)
    bb[:, 0:2] = r2(inputs["query_b"])[0][:, :, 0].T
    bb[:, 2:4] = r2(inputs["out_b"])[0][:, :, 0].T
    bbnd = r2(inputs["band_b"])
    bmod = r2(inputs["mod_b"])
    for l in range(2):
        for i in range(2):
            bb[:, 4 + 2 * l + i] = bbnd[l, i, :, 0]
            bb[:, 8 + 2 * l + i] = bmod[l, i, :, 0]
    bb[:, 12:14] = r2(inputs["hv_b"])[0][:, :, 0].T
    bb[0, 14] = np.asarray(inputs["outl_b"], np.float32).sum()
    shared = {
        "wblob": _bf(wblob),
        "wb64": _bf(wb64),
        "cb64": np.ascontiguousarray(cb64),
        "scb2": np.ascontiguousarray(scb2),
        "bblob": np.ascontiguousarray(bb),
    }
    tokT_full = np.ascontiguousarray(tokens.transpose(0, 2, 1))  # (B, 256, L)

    in_maps = []
    tok_idx = np.arange(CL)
    for c in range(N_CORES):
        m = dict(shared)
        m["x0sT"] = np.ascontiguousarray(grid_sorted[c * QS : (c + 1) * QS].T)
        t0 = t0s[c]
        m["tokT"] = _bf(tokT_full[:, :, t0 : t0 + CL].reshape(B, 2, 128, CL))
        sc = s_sorted[c * QS : (c + 1) * QS].reshape(NQT, QT)
        ta = t0 + tok_idx                                   # absolute token pos
        mk = (ta[None, None, :] >= sc[:, :, None]) & (ta[None, None, :] < sc[:, :, None] + TOPK)
        # (NQT, QT, CC, 128) -> (tok_in_chunk, NQT, CC, QT)
        mk = mk.reshape(NQT, QT, CC, 128).transpose(3, 0, 2, 1)
        m["maskT"] = _bf(mk.reshape(128, NQT * CL))
        in_maps.append(m)

    trace = bool(os.environ.get("KERNEL_TRACE"))
    res = run_bass_kernel_spmd(nc, in_maps, core_ids=list(range(N_CORES)),
                               trace=trace)
    LAST_RESULTS = res
    parts = [res.results[c]["out"] for c in range(N_CORES)]  # each (B, QS)
    cat = np.concatenate(parts, axis=1)                      # (B, Q) sorted order
    out = np.empty((B, Q), np.float32)
    out[:, perm] = cat
    return out.reshape(B, Q, 1).astype(np.float32)
